# revision 12
# baseline (speedup 1.0000x reference)
"""Trainium2 Bass kernel for the BiLstmBlock problem.

Strategy: data-parallel over batch B=8 across 8 NeuronCores (1 sample per
core).  Each core runs both ConvLSTM layers on its sample; the only
cross-core communication is the sync-BN batch-statistics AllReduce
([128,2] per BN layer).

Per-core layout: channels on SBUF partitions, spatial (H*W) on the free
dim.  Each 3x3 conv is 9 shifted K=128 matmuls accumulated in PSUM.
DMA-fed conv inputs (x, hidden states, most weights) are fp32r so the PE
runs at full rate with near-fp32 accuracy; the layer-2 input (produced
on-chip by BN+ReLU) is bf16.  The two LSTM directions are merged on
partitions: gate blocks are [i0;i1],[f0;f1],[g0;g1],[o0;o1], so LSTM
elementwise ops are full-width and the channel-concat output layout
falls out for free.  Hidden-state convs use block-diagonal stacked
weights so one K=128 matmul serves both directions.  Spatial zero
padding is prepared host-side (inputs arrive pre-padded to 130x130).
"""

import sys

for _p in ("/opt/trn_rl_repo",):
    if _p not in sys.path:
        sys.path.insert(0, _p)

import numpy as np

import concourse.bacc as bacc
import concourse.tile as tile
from concourse import mybir
from concourse.bass_utils import run_bass_kernel_spmd

F32 = mybir.dt.float32
F32R = mybir.dt.float32r
BF16 = mybir.dt.bfloat16
AF = mybir.ActivationFunctionType
ALU = mybir.AluOpType
AX = mybir.AxisListType

N_CORES = 8
B, CIN, P, H, W = 8, 128, 64, 128, 128
HW = H * W
WP = W + 2
HP = H + 2
RB = 4                 # output rows per block
NBLK = H // RB
NF = RB * W            # 512 free elems per block
EPS = 1e-5
COUNT = float(B * HW)  # BN reduction count across all cores
STAGES = "ABCE"        # debug: which phases to emit (A, B=bn1, C, E)


def _emit(nc):
    xp_in = nc.dram_tensor("xp", [128, HP * WP], F32R, kind="ExternalInput").ap()
    h1_in = nc.dram_tensor("h1p", [128, HP * WP], F32R, kind="ExternalInput").ap()
    c1_in = nc.dram_tensor("c1", [128, HW], F32, kind="ExternalInput").ap()
    h2_in = nc.dram_tensor("h2p", [128, HP * WP], F32R, kind="ExternalInput").ap()
    c2_in = nc.dram_tensor("c2", [128, HW], F32, kind="ExternalInput").ap()
    wx1_in = nc.dram_tensor("wx1", [128, 9 * 512], F32R, kind="ExternalInput").ap()
    wh1_in = nc.dram_tensor("wh1", [128, 9 * 512], F32R, kind="ExternalInput").ap()
    wx2_in = nc.dram_tensor("wx2", [128, 9 * 512], F32R, kind="ExternalInput").ap()
    wh2_in = nc.dram_tensor("wh2", [128, 9 * 512], F32R, kind="ExternalInput").ap()
    gb_in = nc.dram_tensor("gb", [128, 4], F32, kind="ExternalInput").ap()

    out_o = nc.dram_tensor("out", [128, HW], F32, kind="ExternalOutput").ap()
    h1h_o = nc.dram_tensor("h1h", [128, HW], F32, kind="ExternalOutput").ap()
    h1c_o = nc.dram_tensor("h1c", [128, HW], F32, kind="ExternalOutput").ap()
    h2h_o = nc.dram_tensor("h2h", [128, HW], F32, kind="ExternalOutput").ap()
    h2c_o = nc.dram_tensor("h2c", [128, HW], F32, kind="ExternalOutput").ap()

    cc1_out = nc.dram_tensor("cc1_out", [128, 2], F32, addr_space="Shared").ap()
    cc2_out = nc.dram_tensor("cc2_out", [128, 2], F32, addr_space="Shared").ap()

    xp_v = xp_in.rearrange("p (r c) -> p r c", c=WP)
    h1_v = h1_in.rearrange("p (r c) -> p r c", c=WP)
    h2_v = h2_in.rearrange("p (r c) -> p r c", c=WP)

    with tile.TileContext(nc) as tc:
        with (
            tc.tile_pool(name="wpool", bufs=1) as wpool,
            tc.tile_pool(name="ppool", bufs=1) as ppool,
            tc.tile_pool(name="spool", bufs=1) as spool,
            tc.tile_pool(name="stream", bufs=2) as stream,
            tc.tile_pool(name="ew", bufs=2) as ew,
            tc.tile_pool(name="ew1", bufs=1) as ew1,
            tc.tile_pool(name="psumA", bufs=1, space="PSUM") as psumA,
            tc.tile_pool(name="psumB", bufs=1, space="PSUM") as psumB,
            tc.tile_pool(name="dram", bufs=1, space="DRAM") as dram,
        ):
            # --- persistent tiles ---
            wx1 = wpool.tile([128, 9 * 512], F32R, tag="wx1")
            wh1 = wpool.tile([128, 9 * 512], F32R, tag="wh1")
            wx2 = wpool.tile([128, 9 * 512], F32R, tag="wx2")
            wh2 = wpool.tile([128, 9 * 512], F32R, tag="wh2")
            nc.sync.dma_start(wx1[:], wx1_in[:])
            nc.sync.dma_start(wh1[:], wh1_in[:])
            nc.sync.dma_start(wx2[:], wx2_in[:])
            nc.sync.dma_start(wh2[:], wh2_in[:])

            # layer-1 output, normalized in place later; fp32r, zero-padded
            # (memset can't write fp32r: zero the borders via DVE copies)
            out1p = ppool.tile([128, HP * WP], F32R, tag="out1p")
            o1v = out1p[:].rearrange("p (r c) -> p r c", c=WP)
            zrow = spool.tile([128, WP], F32, tag="zrow")
            nc.vector.memset(zrow[:], 0.0)
            nc.vector.tensor_copy(o1v[:, 0, :], zrow[:])
            nc.vector.tensor_copy(o1v[:, H + 1, :], zrow[:])
            nc.vector.tensor_copy(o1v[:, 1 : H + 1, 0:1], zrow[:, 0:H])
            nc.vector.tensor_copy(o1v[:, 1 : H + 1, W + 1 : W + 2], zrow[:, 0:H])

            st_s1 = spool.tile([128, NBLK], F32, tag="st_s1")
            st_q1 = spool.tile([128, NBLK], F32, tag="st_q1")
            st_s2 = spool.tile([128, NBLK], F32, tag="st_s2")
            st_q2 = spool.tile([128, NBLK], F32, tag="st_q2")
            gbv = spool.tile([128, 4], F32, tag="gbv")
            nc.sync.dma_start(gbv[:], gb_in[:])

            def load_block(rb, src_v, tag):
                """6 padded rows (rb*RB .. rb*RB+5) of a [128,HP,WP] padded
                dram view into a [128,6,WP] tile."""
                t = stream.tile([128, 6, WP], F32R, tag=tag)
                nc.sync.dma_start(t[:], src_v[:, rb * RB : rb * RB + 6, :])
                return t

            def conv_block(psum_pool, ptag, gb, wh_sb, h_rhs, wx_sb, x_rhs):
                """Accumulate the 18 shifted matmuls for one gate block.
                h_rhs/x_rhs: callables (dy, dx) -> rhs AP [128, RB, W]."""
                pg = psum_pool.tile([128, NF], F32, tag=ptag)
                n = 0
                for w_sb, rhs_of in ((wh_sb, h_rhs), (wx_sb, x_rhs)):
                    for dy in range(3):
                        for dx in range(3):
                            tap = dy * 3 + dx
                            lhsT = w_sb[:, tap * 512 + gb * 128 : tap * 512 + (gb + 1) * 128]
                            nc.tensor.matmul(
                                pg[:], lhsT, rhs_of(dy, dx),
                                start=(n == 0), stop=(n == 17),
                            )
                            n += 1
                return pg

            def lstm_block(rb, pgs, c_src, st_s, st_q, c_out, h_out):
                """Gate psums -> c_new, h_new (fp32 tile, returned)."""
                cols = slice(rb * NF, (rb + 1) * NF)
                ti = ew.tile([128, NF], F32, tag="ti")
                tf = ew.tile([128, NF], F32, tag="tf")
                tg = ew.tile([128, NF], F32, tag="tg")
                to = ew.tile([128, NF], F32, tag="to")
                nc.scalar.activation(ti[:], pgs[0][:], AF.Sigmoid)
                nc.scalar.activation(tf[:], pgs[1][:], AF.Sigmoid)
                nc.scalar.activation(tg[:], pgs[2][:], AF.Tanh)
                nc.scalar.activation(to[:], pgs[3][:], AF.Sigmoid)
                ct = stream.tile([128, NF], F32, tag="ct")
                nc.sync.dma_start(ct[:], c_src[:, cols])
                p1 = ew1.tile([128, NF], F32, tag="p1")
                nc.vector.tensor_mul(p1[:], ti[:], tg[:])
                m1 = ew1.tile([128, NF], F32, tag="m1")
                nc.vector.tensor_mul(m1[:], tf[:], ct[:])
                cnew = ew.tile([128, NF], F32, tag="cnew")
                nc.vector.tensor_add(cnew[:], m1[:], p1[:])
                nc.sync.dma_start(c_out[:, cols], cnew[:])
                tcn = ew1.tile([128, NF], F32, tag="m1")
                nc.scalar.activation(tcn[:], cnew[:], AF.Tanh)
                hnew = ew.tile([128, NF], F32, tag="hnew")
                nc.vector.tensor_mul(hnew[:], to[:], tcn[:])
                nc.vector.tensor_reduce(
                    st_s[:, rb : rb + 1], hnew[:], AX.X, ALU.add
                )
                sq = ew1.tile([128, NF], F32, tag="p1")
                nc.scalar.activation(
                    sq[:], hnew[:], AF.Square, accum_out=st_q[:, rb : rb + 1]
                )
                nc.sync.dma_start(h_out[:, cols], hnew[:])
                return hnew

            # ---------------- Phase A: layer 1 ----------------
            for rb in range(NBLK):
                xt = load_block(rb, xp_v, "xt")
                ht = load_block(rb, h1_v, "ht")
                x_rhs = lambda dy, dx, t=xt: t[:, dy : dy + RB, dx : dx + W]
                h_rhs = lambda dy, dx, t=ht: t[:, dy : dy + RB, dx : dx + W]
                pgs = [
                    conv_block(psumA, f"pgA{g}", g, wh1, h_rhs, wx1, x_rhs)
                    for g in range(4)
                ]
                hnew = lstm_block(rb, pgs, c1_in, st_s1, st_q1, h1c_o, h1h_o)
                # stash (bf16) into the padded layer-2 input buffer
                dst = o1v[:, 1 + rb * RB : 1 + rb * RB + RB, 1 : W + 1]
                nc.scalar.copy(dst, hnew[:])

            # ---------------- sync-BN 1 ----------------
            def bn_coeffs(st_s, st_q, cc_out_ap, g_col, b_col, name):
                ccs = spool.tile([128, 2], F32, tag=f"ccs{name}")
                nc.vector.tensor_reduce(ccs[:, 0:1], st_s[:], AX.X, ALU.add)
                nc.vector.tensor_reduce(ccs[:, 1:2], st_q[:], AX.X, ALU.add)
                cci = dram.tile([128, 2], F32, tag=f"cci{name}")
                nc.sync.dma_start(cci[:], ccs[:])
                nc.gpsimd.collective_compute(
                    "AllReduce", ALU.add,
                    replica_groups=[list(range(N_CORES))],
                    ins=[cci.opt()], outs=[cc_out_ap.opt()],
                )
                g = spool.tile([128, 2], F32, tag=f"g{name}")
                nc.sync.dma_start(g[:], cc_out_ap[:])
                mean = spool.tile([128, 1], F32, tag=f"mean{name}")
                ex2 = spool.tile([128, 1], F32, tag=f"ex2{name}")
                nc.vector.tensor_scalar_mul(mean[:], g[:, 0:1], 1.0 / COUNT)
                nc.vector.tensor_scalar_mul(ex2[:], g[:, 1:2], 1.0 / COUNT)
                msq = spool.tile([128, 1], F32, tag=f"msq{name}")
                nc.vector.tensor_mul(msq[:], mean[:], mean[:])
                var = spool.tile([128, 1], F32, tag=f"var{name}")
                nc.vector.tensor_sub(var[:], ex2[:], msq[:])
                veps = spool.tile([128, 1], F32, tag=f"veps{name}")
                nc.vector.tensor_scalar_add(veps[:], var[:], EPS)
                sd = spool.tile([128, 1], F32, tag=f"sd{name}")
                nc.scalar.activation(sd[:], veps[:], AF.Sqrt)
                rv = spool.tile([128, 1], F32, tag=f"rv{name}")
                nc.vector.reciprocal(rv[:], sd[:])
                a = spool.tile([128, 1], F32, tag=f"a{name}")
                am = spool.tile([128, 1], F32, tag=f"am{name}")
                c = spool.tile([128, 1], F32, tag=f"c{name}")
                nc.vector.tensor_mul(a[:], gbv[:, g_col : g_col + 1], rv[:])
                nc.vector.tensor_mul(am[:], a[:], mean[:])
                nc.vector.tensor_sub(c[:], gbv[:, b_col : b_col + 1], am[:])
                return a, c

            if "B" in STAGES:
                a1, c1 = bn_coeffs(st_s1, st_q1, cc1_out, 0, 1, "1")

                # normalize layer-1 output in place (interior only), relu fused
                SR = H // 4
                for s in range(4):
                    rows = o1v[:, 1 + s * SR : 1 + (s + 1) * SR, 1 : W + 1]
                    nc.scalar.activation(
                        rows, rows.bitcast(F32), AF.Relu,
                        bias=c1[:, 0:1], scale=a1[:, 0:1]
                    )

            # ---------------- Phase C: layer 2 ----------------
            for rb in range(NBLK if "C" in STAGES else 0):
                ht = load_block(rb, h2_v, "ht")
                x_rhs = lambda dy, dx, r=rb: o1v[:, r * RB + dy : r * RB + dy + RB, dx : dx + W]
                h_rhs = lambda dy, dx, t=ht: t[:, dy : dy + RB, dx : dx + W]
                pgs = [
                    conv_block(psumB, f"pgB{g}", g, wh2, h_rhs, wx2, x_rhs)
                    for g in range(4)
                ]
                lstm_block(rb, pgs, c2_in, st_s2, st_q2, h2c_o, h2h_o)

            if "E" not in STAGES:
                return
            a2, c2 = bn_coeffs(st_s2, st_q2, cc2_out, 2, 3, "2")

            # ---------------- Phase E: final bn + identity + relu ----------------
            for rb in range(NBLK):
                cols = slice(rb * NF, (rb + 1) * NF)
                o2 = ew.tile([128, NF], F32, tag="ti")
                xe = ew.tile([128, NF], F32, tag="tf")
                nc.sync.dma_start(o2[:], h2h_o[:, cols])
                xsrc = xp_v.bitcast(F32)[:, 1 + rb * RB : 1 + rb * RB + RB, 1 : W + 1]
                nc.sync.dma_start(xe[:], xsrc)
                t1 = ew.tile([128, NF], F32, tag="tg")
                nc.vector.scalar_tensor_tensor(
                    t1[:], o2[:], a2[:, 0:1], xe[:], ALU.mult, ALU.add
                )
                fin = ew.tile([128, NF], F32, tag="to")
                nc.scalar.activation(fin[:], t1[:], AF.Relu, bias=c2[:, 0:1])
                nc.sync.dma_start(out_o[:, cols], fin[:])


_NC_CACHE = None


def _get_nc():
    global _NC_CACHE
    if _NC_CACHE is None:
        nc = bacc.Bacc("TRN2", target_bir_lowering=False, debug=False,
                       num_devices=N_CORES)
        _emit(nc)
        nc.compile()
        _NC_CACHE = nc
    return _NC_CACHE


def _prep_wx(w):
    # w: (2, 4P, C, 3, 3) -> lhsT [C, 9*512]; col = t*512 + gb*128 + d*64 + p
    w = np.asarray(w, np.float32)
    C = w.shape[2]
    v = w.reshape(2, 4, P, C, 3, 3).transpose(3, 4, 5, 1, 0, 2)
    return np.ascontiguousarray(v.reshape(C, 9 * 512))


def _prep_wh(w):
    # w: (2, 4P, P, 3, 3) -> block-diagonal lhsT [2P, 9*512]
    w = np.asarray(w, np.float32)
    out = np.zeros((2 * P, 3, 3, 4, 2, P), np.float32)
    v = w.reshape(2, 4, P, P, 3, 3)
    for d in range(2):
        out[P * d : P * (d + 1), :, :, :, d, :] = v[d].transpose(2, 3, 4, 0, 1)
    return np.ascontiguousarray(out.reshape(2 * P, 9 * 512))


def _pad_hw(a):
    # a: (128, H, W) -> zero-padded (128, HP*WP)
    p = np.zeros((a.shape[0], HP, WP), np.float32)
    p[:, 1 : H + 1, 1 : W + 1] = a
    return p.reshape(a.shape[0], HP * WP)


def _in_maps(x, h1_h, h1_c, h2_h, h2_c, w_ih1, w_hh1, w_ih2, w_hh2,
             gamma1, beta1, gamma2, beta2):
    wx1 = _prep_wx(w_ih1)
    wh1 = _prep_wh(w_hh1)
    wx2 = _prep_wx(w_ih2)
    wh2 = _prep_wh(w_hh2)
    gb = np.ascontiguousarray(
        np.stack([gamma1, beta1, gamma2, beta2], axis=1).astype(np.float32))
    maps = []
    for b in range(N_CORES):
        maps.append({
            "xp": _pad_hw(np.asarray(x[b], np.float32)),
            "h1p": _pad_hw(np.asarray(h1_h[:, b], np.float32).reshape(128, H, W)),
            "c1": np.ascontiguousarray(
                np.asarray(h1_c[:, b], np.float32)).reshape(128, HW),
            "h2p": _pad_hw(np.asarray(h2_h[:, b], np.float32).reshape(128, H, W)),
            "c2": np.ascontiguousarray(
                np.asarray(h2_c[:, b], np.float32)).reshape(128, HW),
            "wx1": wx1, "wh1": wh1, "wx2": wx2, "wh2": wh2, "gb": gb,
        })
    return maps


def kernel(x, h1_h, h1_c, h2_h, h2_c, w_ih1, w_hh1, w_ih2, w_hh2,
           gamma1, beta1, gamma2, beta2):
    nc = _get_nc()
    maps = _in_maps(x, h1_h, h1_c, h2_h, h2_c, w_ih1, w_hh1, w_ih2, w_hh2,
                    gamma1, beta1, gamma2, beta2)
    res = run_bass_kernel_spmd(nc, maps, list(range(N_CORES)))
    r = res.results
    out = np.stack([r[b]["out"].reshape(CIN, H, W) for b in range(N_CORES)])
    h1h = np.stack([r[b]["h1h"].reshape(2, P, H, W) for b in range(N_CORES)], axis=1)
    h1c = np.stack([r[b]["h1c"].reshape(2, P, H, W) for b in range(N_CORES)], axis=1)
    h2h = np.stack([r[b]["h2h"].reshape(2, P, H, W) for b in range(N_CORES)], axis=1)
    h2c = np.stack([r[b]["h2c"].reshape(2, P, H, W) for b in range(N_CORES)], axis=1)
    return (out, h1h, h1c, h2h, h2c)
